# revision 18
# baseline (speedup 1.0000x reference)
"""MultiHeadExternalAttention Trainium2 kernel (v2: fp8 DoubleRow phase 1).

Math (reference):
  h = x @ trans_w.T + trans_b            [B,N,4096] -> heads [B,64,N,64]
  a = h @ lin0_w.T + lin0_b              per-head [B,64,N,64]
  a = softmax(a, axis=N)
  a = a / (1e-10 + a.sum(-1, keepdims))  double norm over j
  o = a @ lin1_w.T + lin1_b
  out = o (merged heads) @ proj_w.T + proj_b

Both tiny linears fold into the big matmuls on the host:
  logits[b,h,n,j] = x[b,n,:] @ fw[h,j,:] + fb[h,j]     fw = lin0_w @ trans_w_h
  out[b,n,c]     += attn[b,h,n,k] * g[h,c,k]           g  = proj_w_h @ lin1_w

Sharding: 8 cores = 4 batches x 2 head-halves (16 head-pairs per core).

Phase 1 runs as fp8e4 DoubleRow matmuls (2 contraction rows/partition, 2x-4x
bf16 rate) with a scale-matched 3-term residual split prepared on the host:
  x_h = fp8(16 x),  x_l = fp8((16 x - x_h) * 16)
  W0  = fp8(256 fw), W1 = fp8(16 fw), W2 = fp8(256 fw - W0)
  psum = x_h.W0 + x_l.W1 + x_h.W2  ~=  4096 * logits  (+- ~0.1%)
exp() evicts psum with scale 1/4096. The double-norm row-reciprocal broadcast
is also a DoubleRow matmul: r32 splits to fp8 hi/lo on DVE, the 0/1 head mask
carries slices (8, 0.5) so sc comes out as 8/rowsum; attn8 = e*rs*sc (bf16,
8x attn) goes to scratch and phase 2 contracts it against host-side g/8.
"""

import sys

if "/opt/trn_rl_repo" not in sys.path:
    sys.path.insert(0, "/opt/trn_rl_repo")

import numpy as np
import ml_dtypes

import concourse.bass as bass
import concourse.bacc as bacc
import concourse.mybir as mybir
import concourse.tile as tile

BF16NP = ml_dtypes.bfloat16
E4NP = ml_dtypes.float8_e4m3
F32 = mybir.dt.float32
BF = mybir.dt.bfloat16
E4 = mybir.dt.float8e4
AF = mybir.ActivationFunctionType
DR = mybir.MatmulPerfMode.DoubleRow

DIM = 512
HEADS = 64
K = 64
B = 4
NTOK = 8192
NCORES = 8
HPC = 16  # head pairs per core


def build_bass(ntok=NTOK, n_hp=HPC, reps=1):
    nc = bacc.Bacc()
    x8h_in = nc.dram_tensor("x8h", [128, 4, ntok], E4, kind="ExternalInput")
    x8l_in = nc.dram_tensor("x8l", [128, 4, ntok], E4, kind="ExternalInput")
    w8 = nc.dram_tensor("w8", [128, n_hp, 3, 2, 2, 128], E4, kind="ExternalInput")
    c1 = nc.dram_tensor("c1", [128, n_hp], F32, kind="ExternalInput")
    p2 = nc.dram_tensor("p2", [128, n_hp, DIM], BF, kind="ExternalInput")
    d2 = nc.dram_tensor("d2", [128, 2, 128], E4, kind="ExternalInput")
    out_p = nc.dram_tensor("out_p", [ntok, DIM], F32, kind="ExternalOutput")

    with tile.TileContext(nc) as tc:
        with tc.tile_pool(name="const", bufs=1) as const, tc.tile_pool(
            name="dramp", bufs=1, space="DRAM"
        ) as dramp:
            scratch = dramp.tile([n_hp, 128, ntok], BF)

            w8_sb = const.tile([128, n_hp, 3, 2, 2, 128], E4)
            c1_sb = const.tile([128, n_hp], F32)
            p2_sb = const.tile([128, n_hp, DIM], BF)
            D2 = const.tile([128, 2, 128], E4)
            xh_sb = const.tile([128, 4, ntok], E4)
            xl_sb = const.tile([128, 4, ntok], E4)

            # per-hp slices, hp0 first, so early matmuls aren't queued
            # behind the bulk of the weight load
            nc.scalar.dma_start(out=w8_sb[:, 0:1], in_=w8[:, 0:1])
            nc.scalar.dma_start(out=c1_sb, in_=c1[:])
            nc.scalar.dma_start(out=D2, in_=d2[:])
            for wc in range(1, n_hp):
                nc.scalar.dma_start(
                    out=w8_sb[:, wc : wc + 1], in_=w8[:, wc : wc + 1]
                )

            for _rep in range(reps):
                run_pipeline(
                    nc, tc, (x8h_in, x8l_in), out_p, scratch, w8_sb, c1_sb,
                    p2_sb, (xh_sb, xl_sb), D2, dramp, ntok, n_hp,
                    p2 if _rep == 0 else None,
                )
    nc.finalize()
    return nc


def run_pipeline(
    nc, tc, x_in, out_p, scratch, w8_sb, c1_sb, p2_sb, x_sb, D2, dramp, ntok,
    n_hp, p2_dram=None,
):
    NT = ntok // 128
    NW = ntok // 512
    Q = min(4, NW)
    NS = NW // Q
    xh_in, xl_in = x_in
    xh_sb, xl_sb = x_sb

    # x load in eighths on two queues so hp0 matmuls start after ~0.5MB
    nq = ntok // 8
    for qt in range(8):
        nc.sync.dma_start(
            out=xh_sb[:, :, nq * qt : nq * (qt + 1)],
            in_=xh_in[:, :, nq * qt : nq * (qt + 1)],
        )
        nc.gpsimd.dma_start(
            out=xl_sb[:, :, nq * qt : nq * (qt + 1)],
            in_=xl_in[:, :, nq * qt : nq * (qt + 1)],
        )
    if p2_dram is not None:
        nc.scalar.dma_start(out=p2_sb, in_=p2_dram[:])

    with tc.tile_pool(name="awin", bufs=4) as awin, tc.tile_pool(
        name="p1p", bufs=2, space="PSUM"
      ) as p1p, tc.tile_pool(
        name="jsp", bufs=1, space="PSUM"
      ) as jsp, tc.tile_pool(
        name="scp", bufs=2, space="PSUM"
      ) as scp, tc.tile_pool(name="ep", bufs=3) as ep, tc.tile_pool(
        name="rcalc", bufs=1
      ) as rcalc, tc.tile_pool(name="rp", bufs=2) as rp, tc.tile_pool(
        name="tmp", bufs=2
      ) as tmp, tc.tile_pool(
        name="small", bufs=3
      ) as small, tc.tile_pool(name="a2p", bufs=2) as a2p, tc.tile_pool(
        name="osp", bufs=2
      ) as osp:
        state = {}

        def gen_step1(hp):
            e = ep.tile([128, ntok], BF, name="e")
            scol = small.tile([128, NW], F32, name="scol")
            state[hp] = [e, None, None]
            # 2 psum tiles (4 DoubleRow windows of 256) per weight pass so
            # each stationary load streams 4x256 columns
            for tp in range(NW // 2):
                pA = p1p.tile([128, 512], F32, name="p1a")
                pB = p1p.tile([128, 512], F32, name="p1b")
                base = 1024 * tp
                combos = [(0, 0), (0, 1), (1, 0), (1, 1), (2, 0), (2, 1)]
                for ci, (term, kp) in enumerate(combos):
                    rhs_src = xl_sb if term == 1 else xh_sb
                    for qq in range(4):
                        tgt = pA if qq < 2 else pB
                        off = 256 * (qq % 2)
                        b0 = base + 256 * qq
                        nc.tensor.matmul(
                            tgt[:, off : off + 256],
                            lhsT=w8_sb[:, hp, term, kp, :, :],
                            rhs=rhs_src[:, 2 * kp : 2 * kp + 2, b0 : b0 + 256],
                            start=(ci == 0 and qq % 2 == 0),
                            stop=(ci == 5),
                            perf_mode=DR,
                            skip_group_check=True,
                        )
                nc.scalar.activation(
                    e[:, base : base + 512],
                    pA,
                    func=AF.Exp,
                    bias=c1_sb[:, hp : hp + 1],
                    scale=1.0 / 4096.0,
                    accum_out=scol[:, 2 * tp : 2 * tp + 1],
                )
                nc.scalar.activation(
                    e[:, base + 512 : base + 1024],
                    pB,
                    func=AF.Exp,
                    bias=c1_sb[:, hp : hp + 1],
                    scale=1.0 / 4096.0,
                    accum_out=scol[:, 2 * tp + 1 : 2 * tp + 2],
                )
                yield
            s1 = small.tile([128, 1], F32, name="s1")
            nc.vector.reduce_sum(s1, scol, axis=mybir.AxisListType.X)
            rs = small.tile([128, 1], F32, name="rs")
            nc.vector.reciprocal(rs, s1)
            Tmh = tmp.tile([128, 66], BF, name="Tmh")
            nc.vector.memset(Tmh, 0.0)
            nc.vector.tensor_copy(Tmh[0:64, 32:33], rs[0:64, :])
            nc.vector.tensor_copy(Tmh[64:128, 33:34], rs[64:128, :])
            state[hp][1] = rs
            state[hp][2] = Tmh

        def emit_jsum(hp):
            e, rs, Tmh = state[hp]
            assert Tmh is not None
            js = jsp.tile([32, 512], F32, name="js")
            for w in range(NW):
                nc.tensor.matmul(
                    js,
                    lhsT=Tmh[:, 32 - 2 * w : 64 - 2 * w],
                    rhs=e[:, 512 * w : 512 * (w + 1)],
                    start=(w == 0),
                    stop=(w == NW - 1),
                )
            r32 = rcalc.tile([32, 512], F32, name="r32")
            nc.vector.reciprocal(r32[0 : 2 * NW, :], js[0 : 2 * NW, :])
            # fp8 hi/lo split of the row reciprocals (DoubleRow bcast operand)
            r8h = rcalc.tile([32, 512], E4, name="r8h")
            nc.vector.tensor_copy(r8h[0 : 2 * NW, :], r32[0 : 2 * NW, :])
            nc.vector.tensor_sub(
                r32[0 : 2 * NW, :], r32[0 : 2 * NW, :], r8h[0 : 2 * NW, :]
            )
            r8l = rcalc.tile([32, 512], E4, name="r8l")
            nc.vector.tensor_scalar_mul(
                r8l[0 : 2 * NW, :], r32[0 : 2 * NW, :], 16.0
            )
            # bounce through DRAM to reshape [2w+g, n] -> quadrant layout
            rdram = dramp.tile([32, 2, 512], E4, name="rdram", bufs=2)
            nc.sync.dma_start(out=rdram[0 : 2 * NW, 0, :], in_=r8h[0 : 2 * NW, :])
            nc.sync.dma_start(out=rdram[0 : 2 * NW, 1, :], in_=r8l[0 : 2 * NW, :])
            # pack window w at partitions 32*(w%Q)+{0,1}, free slot w//Q
            r2a = rp.tile([128, NS, 2, 512], E4, name="r2a")
            rv = rdram[0 : 2 * NW, :, :].rearrange(
                "(s q g) h n -> q g s h n", q=Q, g=2
            )
            for q in range(Q):
                nc.sync.dma_start(
                    out=r2a[32 * q : 32 * q + 2, :, :, :], in_=rv[q]
                )
            state[hp] = [e, rs, r2a]

        def gen_bcast(hp):
            e, rs, r2a = state.pop(hp)
            for w in range(NW):
                q = w % Q
                sc = scp.tile([128, 512], F32, name="sc")
                for h2 in range(2):
                    nc.tensor.matmul(
                        sc[:, 256 * h2 : 256 * h2 + 256],
                        lhsT=D2[32 * q : 32 * q + 2, :, :],
                        rhs=r2a[
                            32 * q : 32 * q + 2, w // Q, :,
                            256 * h2 : 256 * h2 + 256,
                        ],
                        start=(h2 == 0),
                        stop=True,
                        perf_mode=DR,
                        tile_position=(32 * q, 0),
                        skip_group_check=True,
                    )
                attn = awin.tile([128, 512], BF, name="attn")
                nc.vector.scalar_tensor_tensor(
                    out=attn,
                    in0=e[:, 512 * w : 512 * (w + 1)],
                    scalar=rs,
                    in1=sc,
                    op0=mybir.AluOpType.mult,
                    op1=mybir.AluOpType.mult,
                )
                nc.gpsimd.dma_start(
                    out=scratch[hp, :, 512 * w : 512 * (w + 1)], in_=attn
                )
                if w % 2 == 1:
                    yield

        # phase 2: out[n,c] = sum_hp attn8_hp[:, chunk].T @ (p2_hp/8)
        def gen_phase2():
            for i in range(NT // 2):
                a2 = a2p.tile([128, n_hp, 256], BF, name="a2")
                nc.scalar.dma_start(
                    out=a2,
                    in_=scratch[
                        :, :, 256 * i : 256 * (i + 1)
                    ].rearrange("h p n -> p h n"),
                )
                for half, pname in ((0, "p1a"), (1, "p1b")):
                    po = p1p.tile([128, 512], F32, name=pname)
                    for hpi in range(n_hp):
                        nc.tensor.matmul(
                            po,
                            lhsT=a2[:, hpi, 128 * half : 128 * half + 128],
                            rhs=p2_sb[:, hpi, :],
                            start=(hpi == 0),
                            stop=(hpi == n_hp - 1),
                        )
                    osb = osp.tile([128, DIM], F32, name="osb")
                    nc.scalar.activation(osb, po, func=AF.Copy)
                    nc.scalar.dma_start(
                        out=out_p[256 * i + 128 * half : 256 * i + 128 * half + 128, :],
                        in_=osb,
                    )
                yield

        p2g = gen_phase2()
        for hp in range(n_hp + 2):
            if 1 <= hp <= n_hp:
                emit_jsum(hp - 1)
            s1g = gen_step1(hp) if hp < n_hp else None
            bcg = gen_bcast(hp - 2) if hp >= 2 else None
            tail = hp >= n_hp + 1
            while s1g is not None or bcg is not None:
                if s1g is not None and next(s1g, "END") == "END":
                    s1g = None
                if bcg is not None and next(bcg, "END") == "END":
                    bcg = None
                if tail:
                    next(p2g, None)
        for _ in p2g:
            pass


def fuse_weights(inputs):
    tw = np.asarray(inputs["trans_w"], np.float64)  # [4096, 512]
    tb = np.asarray(inputs["trans_b"], np.float64)  # [4096]
    l0w = np.asarray(inputs["lin0_w"], np.float64)  # [64, 64]
    l0b = np.asarray(inputs["lin0_b"], np.float64)
    l1w = np.asarray(inputs["lin1_w"], np.float64)
    l1b = np.asarray(inputs["lin1_b"], np.float64)
    pw = np.asarray(inputs["proj_w"], np.float64)  # [512, 4096]
    pb = np.asarray(inputs["proj_b"], np.float64)

    tw3 = tw.reshape(HEADS, K, DIM)
    tb2 = tb.reshape(HEADS, K)
    fw = np.einsum("jk,hkc->hjc", l0w, tw3)  # [64, 64, 512]
    fb = l0b[None, :] + np.einsum("jk,hk->hj", l0w, tb2)  # [64, 64]
    pw3 = pw.reshape(DIM, HEADS, K).transpose(1, 0, 2)  # [h, c, j]
    g = np.einsum("hcj,jk->hck", pw3, l1w)  # [64, 512, 64]
    cb = pb + np.einsum("hcj,j->c", pw3, l1b)  # [512]
    return fw, fb, g, cb


def prep_x(x):
    """Per-batch transposed fp8 hi/lo splits: [B][128, 4, N] each."""
    xs_h, xs_l = [], []
    for b in range(x.shape[0]):
        xt = np.ascontiguousarray(x[b].T, np.float32)  # [512, N]
        x16 = (xt * 16.0).reshape(4, 128, -1).transpose(1, 0, 2)  # [128,4,N]
        xh = x16.astype(E4NP)
        xl = ((x16 - xh.astype(np.float32)) * 16.0).astype(E4NP)
        xs_h.append(np.ascontiguousarray(xh))
        xs_l.append(np.ascontiguousarray(xl))
    return xs_h, xs_l


def make_core_inputs(xs_h, xs_l, fw, fb, g, b, gg, n_hp=HPC):
    """Inputs for the core handling batch b, head half gg (heads 32*gg..+32)."""
    h0 = (HEADS // 2) * gg
    w8 = np.empty((128, n_hp, 3, 2, 2, 128), E4NP)
    c1 = np.empty((128, n_hp), np.float32)
    p2 = np.empty((128, n_hp, DIM), BF16NP)
    for hp in range(n_hp):
        ha, hb = h0 + 2 * hp, h0 + 2 * hp + 1
        blk = np.concatenate([fw[ha], fw[hb]], axis=0)  # [128 j2, 512 c]
        bT = np.ascontiguousarray(blk.T, np.float64)  # [512 c, 128 j2]
        W0 = (bT * 256.0).astype(E4NP)
        W1 = (bT * 16.0).astype(E4NP)
        W2 = (bT * 256.0 - W0.astype(np.float64)).astype(E4NP)
        # [c, j2] -> [kp 2, slice 2, ci 128, j2] -> [ci, kp, slice, j2]
        for t, W in enumerate((W0, W1, W2)):
            w8[:, hp, t] = W.reshape(2, 2, 128, 128).transpose(2, 0, 1, 3)
        c1[:, hp] = np.concatenate([fb[ha], fb[hb]]).astype(np.float32)
        # p2[g2*64+k, hp, c] = g[head, c, k] / 8  (attn8 carries 8x attn)
        p2[0:64, hp, :] = (g[ha].T / 8.0).astype(BF16NP)
        p2[64:128, hp, :] = (g[hb].T / 8.0).astype(BF16NP)
    d2 = np.zeros((128, 2, 128), E4NP)
    for q in range(4):
        d2[32 * q + 0, 0, 0:64] = 8.0
        d2[32 * q + 1, 0, 64:128] = 8.0
        d2[32 * q + 0, 1, 0:64] = 0.5
        d2[32 * q + 1, 1, 64:128] = 0.5
    return {
        "x8h": xs_h[b],
        "x8l": xs_l[b],
        "w8": w8,
        "c1": c1,
        "p2": p2,
        "d2": d2,
    }


_NC_CACHE = None
LAST_RESULTS = None


def kernel(**inputs):
    global _NC_CACHE, LAST_RESULTS
    from concourse.bass_utils import run_bass_kernel_spmd

    x = np.asarray(inputs["x"], np.float32)
    fw, fb, g, cb = fuse_weights(inputs)
    xs_h, xs_l = prep_x(x)

    if _NC_CACHE is None:
        _NC_CACHE = build_bass()
    nc = _NC_CACHE

    in_maps = []
    for c in range(NCORES):
        b, gg = c // 2, c % 2
        in_maps.append(make_core_inputs(xs_h, xs_l, fw, fb, g, b, gg))

    res = run_bass_kernel_spmd(nc, in_maps, list(range(NCORES)))
    LAST_RESULTS = res

    out = np.empty((B, NTOK, DIM), np.float32)
    cbf = cb.astype(np.float32)
    for b in range(B):
        out[b] = res.results[2 * b]["out_p"] + res.results[2 * b + 1]["out_p"]
        out[b] += cbf[None, :]
    return out
